# revision 24
# baseline (speedup 1.0000x reference)
import sys
import contextlib

sys.path.insert(0, "/opt/trn_rl_repo")

import numpy as np

import concourse.bass as bass
import concourse.mybir as mybir
import concourse.tile as tile
from concourse import bacc
from concourse.bass_utils import run_bass_kernel_spmd

# Problem constants (nn_DT_GCN_Lite): hardcoded per harness contract.
N_NODES = 100000
N_EDGES = 1000000
IN_CH = 64
OUT_CH = 128
N_CORES = 8

WINDOW = 128                       # nodes per destination window
WINDOWS_PER_CORE = 98              # 98 * 128 = 12544 nodes per core
NODES_PER_CORE = WINDOWS_PER_CORE * WINDOW
N_WINDOWS = WINDOWS_PER_CORE * N_CORES  # 784 global windows

P = 128                            # edges per block (one partition each)
CHUNK = 25000                      # nodes per x-chunk (<=32768 for int16)
N_CHUNKS = 4
G = 7                              # windows per gather/one-hot group (98 = 14*7)
N_GROUPS = WINDOWS_PER_CORE // G
MAX_NI = 896                       # SWDGE ring limit per sub-gather (see SWDGE_SCRATCH)
SWDGE_SCRATCH = 16384

FP = mybir.dt.float32
HF = mybir.dt.float16
NP_FP = np.float32
NP_HF = np.float16


def _layout(cap_wc):
    """Static layout from cap_wc [W, C] (padded per-bucket edge counts,
    multiples of 128, shared across cores). Groups are G consecutive
    window positions. Msg/one-hot columns within a group are ordered
    (ch, w, j); gathers are per (group, chunk), ring-split at MAX_NI."""
    W = cap_wc.shape[0]
    nblk_wc = cap_wc // P                               # [W, C]
    nblk_w = nblk_wc.sum(axis=1)
    NBLK = int(nblk_wc.sum())

    groups = [list(range(g * G, (g + 1) * G)) for g in range(W // G)]
    msg_off_gchw = {}                # (w, ch) -> group-local column
    msg_cols_g = []                  # blocks per group
    grp_base = []                    # group -> global column base
    gth = []                         # (gi, ch, ni, sidx_off, moff)
    sidx = 0
    acc = 0
    for gi, ws in enumerate(groups):
        grp_base.append(acc)
        off = 0
        for ch in range(N_CHUNKS):
            ni = int(cap_wc[ws, ch].sum())
            done = 0
            while done < ni:
                sub = min(MAX_NI, ni - done)
                gth.append((gi, ch, sub, sidx, off + done // P))
                sidx += sub // 16
                done += sub
            for w in ws:
                msg_off_gchw[(w, ch)] = off
                off += int(nblk_wc[w, ch])
        msg_cols_g.append(off)
        acc += off
    return dict(
        nblk_wc=nblk_wc, nblk_w=nblk_w, NBLK=NBLK,
        groups=groups, msg_off_gchw=msg_off_gchw, msg_cols_g=msg_cols_g,
        grp_base=grp_base, gth=gth, SIDX=sidx, MAXBG=max(msg_cols_g),
    )


def _dma_gather_raw(eng, out_ap, in_ap, idxs_ap, num_idxs, elem_size, elem_step,
                    queue_num):
    """dma_gather with elem payload < 256B (stride must still be 256B-aligned;
    the bass-level %256 elem assert is a transpose-path restriction)."""
    assert idxs_ap.dtype == mybir.dt.int16
    assert in_ap.dtype == out_ap.dtype
    stride_bytes = elem_step * mybir.dt.size(in_ap.dtype)
    assert stride_bytes % 256 == 0
    _in_ap = eng.lower_ap_dma(in_ap, for_custom_bir_dma=True)
    return eng.add_instruction(
        mybir.InstDMAGatherAnt(
            name=eng.bass.get_next_instruction_name(),
            ins=[
                *_in_ap,
                eng.lower_ap(idxs_ap),
                eng.lower_val_access(eng.to_reg(num_idxs)),
            ],
            outs=[eng.lower_ap(out_ap)],
            transpose=False,
            num_idxs=num_idxs,
            elem_size=elem_size,
            stride_bytes_256=stride_bytes // 256,
            gen_mode=0,
            single_packet=True,
            queue_num=queue_num,
            sbuf_tokens_per_rank=0,
            sbuf_free_dim_per_rank=0,
            sbuf_free_dim_pad_per_rank=0,
            sbuf_byte_offset=0,
        )
    )


def build_nc(cap_wc, repeat=1, no_gather=False, no_dve=False, no_pe=False,
             gather_mode="fp16"):
    L = _layout(cap_wc)
    NBLK, SIDX, MAXBG = L["NBLK"], L["SIDX"], L["MAXBG"]
    nc = bacc.Bacc("TRN2", target_bir_lowering=False, num_swdge_queues=4,
                   dynamic_dma_scratch_size=SWDGE_SCRATCH)

    if gather_mode == "fp32":
        x_d = nc.dram_tensor("x", [N_NODES, IN_CH], FP, kind="ExternalInput")
        MSG_DT = FP
    elif gather_mode == "fp32x2":
        x_d = nc.dram_tensor("x2", [N_NODES // 2, 2 * IN_CH], FP, kind="ExternalInput")
        MSG_DT = FP
    elif gather_mode == "fp16":
        x_d = nc.dram_tensor("xh", [N_NODES, 2 * IN_CH], HF, kind="ExternalInput")
        MSG_DT = HF
    else:
        raise ValueError(gather_mode)
    idx_d = nc.dram_tensor("idx16", [P, SIDX], mybir.dt.int16, kind="ExternalInput")
    rowl_d = nc.dram_tensor("rowl", [P, NBLK], HF, kind="ExternalInput")
    wtsm_d = nc.dram_tensor("wtsm", [P, NBLK], FP, kind="ExternalInput")
    iota_d = nc.dram_tensor("iotar", [P, WINDOW * MAXBG], HF, kind="ExternalInput")
    wt_d = nc.dram_tensor("wt", [IN_CH, OUT_CH], HF, kind="ExternalInput")
    bias4_d = nc.dram_tensor("bias4", [1, 4 * OUT_CH], HF, kind="ExternalInput")
    out_d = nc.dram_tensor("out", [NODES_PER_CORE, OUT_CH], FP, kind="ExternalOutput")

    with tile.TileContext(nc) as tc:
        with (
            tc.tile_pool(name="const", bufs=1) as const_pool,
            tc.tile_pool(name="msg", bufs=4) as msg_pool,
            tc.tile_pool(name="msgh", bufs=3) as msgh_pool,
            tc.tile_pool(name="oh", bufs=2) as oh_pool,
            tc.tile_pool(name="aggp", bufs=2, space="PSUM") as aggp_pool,
            tc.tile_pool(name="aggs", bufs=2) as aggs_pool,
            tc.tile_pool(name="outp", bufs=2, space="PSUM") as outp_pool,
            tc.tile_pool(name="outs", bufs=2) as outs_pool,
        ):
            idx_sb = const_pool.tile([P, SIDX], mybir.dt.int16)
            rowl_sb = const_pool.tile([P, NBLK], HF)
            wtsm_sb = const_pool.tile([P, NBLK], FP)
            iota_sb = const_pool.tile([P, WINDOW * MAXBG], HF)
            wt_sb = const_pool.tile([IN_CH, OUT_CH], HF)
            bias4_sb = const_pool.tile([1, 4 * OUT_CH], HF)
            ones_sb = const_pool.tile([1, OUT_CH], HF)

            nc.sync.dma_start(idx_sb[:], idx_d[:])
            nc.sync.dma_start(rowl_sb[:], rowl_d[:])
            nc.sync.dma_start(wtsm_sb[:], wtsm_d[:])
            nc.sync.dma_start(iota_sb[:], iota_d[:])
            nc.sync.dma_start(wt_sb[:], wt_d[:])
            nc.sync.dma_start(bias4_sb[:], bias4_d[:])
            nc.vector.memset(ones_sb[:], 1.0)

            loop_cm = tc.For_i(0, repeat, 1) if repeat > 1 else contextlib.nullcontext()
            with loop_cm:
                for gi, ws in enumerate(L["groups"]):
                    nblk_g = L["msg_cols_g"][gi]
                    gb = L["grp_base"][gi]
                    msg_w = 2 * IN_CH if gather_mode == "fp32x2" else IN_CH
                    msg = msg_pool.tile([P, MAXBG * msg_w], MSG_DT, tag="msg")
                    if not no_dve:
                        msgh = msgh_pool.tile([P, MAXBG * IN_CH], HF, tag="msgh")
                        oh = oh_pool.tile([P, WINDOW * MAXBG], HF, tag="oh")

                    # one-hot for the whole group, [d, blk] free layout so all
                    # operands are 2-byte packed innermost (DVE 2x_1p mode):
                    # oh[e, d*nblk_g + m] = (rowl[e, gb+m] == d)
                    if not no_dve:
                        iap = iota_sb[:]
                        rap = rowl_sb[:]
                        oap = oh[:]
                        nc.vector.tensor_tensor(
                            out=bass.AP(
                                oap.tensor, oap.offset,
                                [oap.ap[0], [nblk_g, WINDOW], [1, nblk_g]],
                            ),
                            in0=bass.AP(
                                iap.tensor, iap.offset,
                                [iap.ap[0], [MAXBG, WINDOW], [1, nblk_g]],
                            ),
                            in1=bass.AP(
                                rap.tensor, rap.offset + gb,
                                [rap.ap[0], [0, WINDOW], [1, nblk_g]],
                            ),
                            op=mybir.AluOpType.is_equal,
                        )

                    # gathers (ring-limited) per (group, chunk)
                    if not no_gather:
                        for (gi2, ch, ni, sidx_off, moff) in L["gth"]:
                            if gi2 != gi or ni == 0:
                                continue
                            nblk_s = ni // P
                            ix = idx_sb[:, sidx_off : sidx_off + ni // 16]
                            qn = (sidx_off // 56) % 4
                            lo, hi = ch * CHUNK, min((ch + 1) * CHUNK, N_NODES)
                            if gather_mode == "fp32":
                                nc.gpsimd.dma_gather(
                                    out_ap=msg[:, moff * IN_CH : (moff + nblk_s) * IN_CH]
                                    .rearrange("p (k d) -> p k d", k=nblk_s),
                                    in_ap=x_d[lo:hi, :],
                                    idxs_ap=ix, num_idxs=ni, num_idxs_reg=ni,
                                    elem_size=IN_CH, queue_num=qn,
                                )
                            elif gather_mode == "fp32x2":
                                nc.gpsimd.dma_gather(
                                    out_ap=msg[:, moff * msg_w : (moff + nblk_s) * msg_w]
                                    .rearrange("p (k d) -> p k d", k=nblk_s),
                                    in_ap=x_d[lo // 2 : hi // 2, :],
                                    idxs_ap=ix, num_idxs=ni, num_idxs_reg=ni,
                                    elem_size=2 * IN_CH, queue_num=qn,
                                )
                            else:  # fp16: 128B payload, 256B source stride
                                _dma_gather_raw(
                                    nc.gpsimd,
                                    out_ap=msg[:, moff * IN_CH : (moff + nblk_s) * IN_CH]
                                    .rearrange("p (k d) -> p k d", k=nblk_s),
                                    in_ap=x_d[lo:hi, 0:IN_CH],
                                    idxs_ap=ix, num_idxs=ni,
                                    elem_size=IN_CH, elem_step=2 * IN_CH,
                                    queue_num=qn,
                                )
                    else:
                        # full-tile init on Pool (keeps Pool comparably busy,
                        # avoids unwritten-subtile reads in msg-mult)
                        nc.gpsimd.memset(msg[:], 0.0)

                    # msgh = msg * w (fp32 -> fp16), one op per (group, chunk)
                    # region so early chunks feed PE while later ones gather
                    if not no_dve:
                        for ch in range(N_CHUNKS):
                            c0 = L["msg_off_gchw"][(ws[0], ch)]
                            ncols = int(sum(L["nblk_wc"][w, ch] for w in ws))
                            if ncols == 0:
                                continue
                            wts_ap = wtsm_sb[:, gb + c0 : gb + c0 + ncols]
                            nc.vector.tensor_tensor(
                                out=msgh[:, c0 * IN_CH : (c0 + ncols) * IN_CH]
                                .rearrange("p (k d) -> p k d", k=ncols),
                                in0=msg[:, c0 * IN_CH : (c0 + ncols) * IN_CH]
                                .rearrange("p (k d) -> p k d", k=ncols),
                                in1=bass.AP(
                                    wts_ap.tensor, wts_ap.offset,
                                    [wts_ap.ap[0], [wts_ap.ap[1][0], ncols], [0, IN_CH]],
                                ),
                                op=mybir.AluOpType.mult,
                            )


                    # aggregation: per window accumulate blocks into a shared
                    # 4-window / 3-window PSUM tile (one bank each)
                    if not no_pe:
                        agg_a = aggp_pool.tile([IN_CH, 4 * WINDOW], FP, tag="agg_a")
                        agg_b = aggp_pool.tile([IN_CH, 3 * WINDOW], FP, tag="agg_b")
                    for wi, w in enumerate(ws if not no_pe else []):
                        tgt = (
                            agg_a[:, wi * WINDOW : (wi + 1) * WINDOW]
                            if wi < 4
                            else agg_b[:, (wi - 4) * WINDOW : (wi - 3) * WINDOW]
                        )
                        cols = []
                        for ch in range(N_CHUNKS):
                            m0 = L["msg_off_gchw"][(w, ch)]
                            cols.extend(range(m0, m0 + int(L["nblk_wc"][w, ch])))
                        ohap = oh[:]
                        for k, m in enumerate(cols):
                            nc.tensor.matmul(
                                tgt,
                                lhsT=msgh[:, m * IN_CH : (m + 1) * IN_CH],
                                rhs=bass.AP(
                                    ohap.tensor, ohap.offset + m,
                                    [ohap.ap[0], [nblk_g, WINDOW]],
                                ),
                                start=(k == 0),
                                stop=(k == len(cols) - 1),
                            )
                    aggs_a = aggs_pool.tile([IN_CH, 4 * WINDOW], HF, tag="aggs_a")
                    aggs_b = aggs_pool.tile([IN_CH, 3 * WINDOW], HF, tag="aggs_b")
                    if not no_pe:
                        nc.scalar.copy(aggs_a[:], agg_a[:])
                        nc.scalar.copy(aggs_b[:], agg_b[:])
                    else:
                        nc.vector.memset(aggs_a[:], 0.0)
                        nc.vector.memset(aggs_b[:], 0.0)

                    # out = agg^T @ W^T + b; bias preloads PSUM via ones matmul
                    op_a = outp_pool.tile([WINDOW, 4 * OUT_CH], FP, tag="op_a")
                    op_b = outp_pool.tile([WINDOW, 3 * OUT_CH], FP, tag="op_b")
                    nc.tensor.matmul(
                        op_a[:], lhsT=ones_sb[:], rhs=bias4_sb[:],
                        start=True, stop=False,
                    )
                    nc.tensor.matmul(
                        op_b[:], lhsT=ones_sb[:], rhs=bias4_sb[:, : 3 * OUT_CH],
                        start=True, stop=False,
                    )
                    for wi in range(G):
                        src = aggs_a if wi < 4 else aggs_b
                        dst = op_a if wi < 4 else op_b
                        j = wi if wi < 4 else wi - 4
                        last = (wi == 3) if wi < 4 else (wi == G - 1)
                        nc.tensor.matmul(
                            dst[:, j * OUT_CH : (j + 1) * OUT_CH],
                            lhsT=src[:, j * WINDOW : (j + 1) * WINDOW],
                            rhs=wt_sb[:],
                            start=False,
                            stop=last,
                        )
                    outs_a = outs_pool.tile([WINDOW, 4 * OUT_CH], FP, tag="outs_a")
                    outs_b = outs_pool.tile([WINDOW, 3 * OUT_CH], FP, tag="outs_b")
                    nc.scalar.copy(outs_a[:], op_a[:])
                    nc.scalar.copy(outs_b[:], op_b[:])
                    # SBUF -> HBM: rows (pos*128+d), cols o
                    r0 = gi * G * WINDOW
                    oda = out_d[r0 : r0 + 4 * WINDOW, :]
                    nc.sync.dma_start(
                        bass.AP(
                            oda.tensor, oda.offset,
                            [[OUT_CH, WINDOW], [WINDOW * OUT_CH, 4], [1, OUT_CH]],
                        ),
                        outs_a[:].rearrange("p (k d) -> p k d", k=4),
                    )
                    odb = out_d[r0 + 4 * WINDOW : r0 + G * WINDOW, :]
                    nc.sync.dma_start(
                        bass.AP(
                            odb.tensor, odb.offset,
                            [[OUT_CH, WINDOW], [WINDOW * OUT_CH, 3], [1, OUT_CH]],
                        ),
                        outs_b[:].rearrange("p (k d) -> p k d", k=3),
                    )
    nc.compile()
    return nc


def preprocess(x, edge_index, edge_weight):
    """Bucket edges by (window, chunk); permute windows across cores so
    same-position windows have matched counts (minimizes shared-cap
    padding). Returns per-core input maps, cap_wc, and perm [8, 98]."""
    row = np.asarray(edge_index[0], dtype=np.int64)
    col = np.asarray(edge_index[1], dtype=np.int64)
    wts = np.asarray(edge_weight, dtype=NP_FP)

    gwin = row >> 7
    ch = col // CHUNK
    key = gwin * N_CHUNKS + ch
    order = np.argsort(key, kind="stable")
    row_s, col_s, w_s = row[order], col[order], wts[order]

    counts_gc = np.bincount(key, minlength=N_WINDOWS * N_CHUNKS).reshape(
        N_WINDOWS, N_CHUNKS
    )
    starts = np.zeros(N_WINDOWS * N_CHUNKS + 1, dtype=np.int64)
    np.cumsum(counts_gc.reshape(-1), out=starts[1:])

    # rank windows by total count desc; slot k = ranked[8k:8k+8] across cores
    wtot = counts_gc.sum(axis=1)
    ranked = np.argsort(-wtot, kind="stable")
    slots = ranked.reshape(WINDOWS_PER_CORE, N_CORES)          # [slot, core]
    slot_cap = -(-counts_gc[slots].max(axis=1) // P) * P       # [slot, C]
    for s in range(WINDOWS_PER_CORE):
        if slot_cap[s].sum() == 0:
            slot_cap[s, 0] = P
    # deal slots serpentine into N_GROUPS groups of G to balance group sizes,
    # then relabel so each group's positions are contiguous
    sc_tot = slot_cap.sum(axis=1)
    sorder = np.argsort(-sc_tot, kind="stable")
    gassign = [[] for _ in range(N_GROUPS)]
    for i, s in enumerate(sorder):
        r = i // N_GROUPS
        gi = i % N_GROUPS if r % 2 == 0 else N_GROUPS - 1 - (i % N_GROUPS)
        gassign[gi].append(s)
    pos_of_slot = np.empty(WINDOWS_PER_CORE, dtype=np.int64)
    for gi in range(N_GROUPS):
        for j, s in enumerate(gassign[gi]):
            pos_of_slot[s] = gi * G + j
    cap_wc = np.empty_like(slot_cap)
    cap_wc[pos_of_slot] = slot_cap                              # [pos, C]
    perm = np.empty((N_CORES, WINDOWS_PER_CORE), dtype=np.int64)
    for s in range(WINDOWS_PER_CORE):
        perm[:, pos_of_slot[s]] = slots[s]

    L = _layout(cap_wc)
    NBLK, SIDX, MAXBG = L["NBLK"], L["SIDX"], L["MAXBG"]
    nblk_wc = L["nblk_wc"]

    # iota_rep[e, d*MAXBG + j] = d  (shared by all cores)
    iota_rep = np.broadcast_to(
        np.repeat(np.arange(WINDOW, dtype=NP_HF), MAXBG)[None, :], (P, WINDOW * MAXBG)
    ).copy()

    in_maps = []
    for c in range(N_CORES):
        rowl_a = np.full((P, NBLK), -1.0, dtype=NP_HF)   # msg-column order
        wtsm_a = np.zeros((P, NBLK), dtype=NP_FP)
        idx_blk = np.zeros((P, NBLK), dtype=np.int16)
        for pos in range(WINDOWS_PER_CORE):
            g = perm[c, pos]
            gi = pos // G
            gbase = L["grp_base"][gi]
            for chn in range(N_CHUNKS):
                nblk = int(nblk_wc[pos, chn])
                if nblk == 0:
                    continue
                k = g * N_CHUNKS + chn
                s, e = starts[k], starts[k + 1]
                cnt = e - s
                cap = nblk * P
                m0 = gbase + L["msg_off_gchw"][(pos, chn)]
                re_ = np.full((cap,), -1.0, dtype=NP_HF)
                we = np.zeros((cap,), dtype=NP_FP)
                ce = np.zeros((cap,), dtype=np.int16)
                re_[:cnt] = (row_s[s:e] - g * WINDOW).astype(NP_HF)
                we[:cnt] = w_s[s:e]
                ce[:cnt] = (col_s[s:e] - chn * CHUNK).astype(np.int16)
                # edge i -> partition i%128, block i//128
                rowl_a[:, m0 : m0 + nblk] = re_.reshape(nblk, P).T
                wtsm_a[:, m0 : m0 + nblk] = we.reshape(nblk, P).T
                idx_blk[:, m0 : m0 + nblk] = ce.reshape(nblk, P).T

        # idx16: wrapped indices per (group, chunk) gather stream
        idx16 = np.zeros((P, SIDX), dtype=np.int16)
        emitted = set()
        for (gi, chn, ni_sub, sidx_off, moff) in L["gth"]:
            if ni_sub == 0 or (gi, chn) in emitted:
                continue
            emitted.add((gi, chn))
            ws = L["groups"][gi]
            ni = int(cap_wc[ws, chn].sum())
            gbase = L["grp_base"][gi]
            c0 = L["msg_off_gchw"][(ws[0], chn)]
            ncols = int(sum(nblk_wc[w, chn] for w in ws))
            # stream = this (g,ch) region's edges in (block, partition) order
            stream = (
                idx_blk[:, gbase + c0 : gbase + c0 + ncols].T.reshape(-1)
            )
            assert stream.shape[0] == ni
            wrapped = stream.reshape(ni // 16, 16).T
            idx16[:, sidx_off : sidx_off + ni // 16] = np.tile(wrapped, (8, 1))

        in_maps.append({"idx16": idx16, "rowl": rowl_a, "wtsm": wtsm_a,
                        "iotar": iota_rep})
    return in_maps, cap_wc, perm


_CACHE = {}


def kernel(x, edge_index, edge_weight, W, b):
    x = np.asarray(x, dtype=NP_FP)
    W = np.asarray(W, dtype=NP_FP)
    bb = np.asarray(b, dtype=NP_FP)

    in_maps, cap_wc, perm = preprocess(x, edge_index, edge_weight)

    key = cap_wc.tobytes()
    if key not in _CACHE:
        _CACHE[key] = build_nc(cap_wc)
    nc = _CACHE[key]

    wt = np.ascontiguousarray(W.T).astype(NP_HF)
    bias4 = np.tile(bb.astype(NP_HF), 4).reshape(1, 4 * OUT_CH)
    xh = np.zeros((N_NODES, 2 * IN_CH), NP_HF)
    xh[:, :IN_CH] = x
    for c in range(N_CORES):
        in_maps[c]["xh"] = xh
        in_maps[c]["wt"] = wt
        in_maps[c]["bias4"] = bias4

    res = run_bass_kernel_spmd(nc, in_maps, core_ids=list(range(N_CORES)))
    # unpermute: core c position i -> global window perm[c][i]
    full = np.empty((N_WINDOWS * WINDOW, OUT_CH), dtype=NP_FP)
    for c in range(N_CORES):
        o = res.results[c]["out"].reshape(WINDOWS_PER_CORE, WINDOW, OUT_CH)
        full.reshape(N_WINDOWS, WINDOW, OUT_CH)[perm[c]] = o
    return full[:N_NODES]
